# revision 4
# baseline (speedup 1.0000x reference)
import sys

sys.path.insert(0, "/opt/trn_rl_repo")

import numpy as np

B, S, D, H, DK, DV = 2, 2048, 1024, 16, 64, 64
N_CORES = 8
LN_EPS = 1e-5
NEG = -1.0e9

_cache = {}


def _build(general: bool):
    import concourse.bacc as bacc
    import concourse.tile as tile
    from concourse import mybir

    F32 = mybir.dt.float32
    F32R = mybir.dt.float32r
    BF16 = mybir.dt.bfloat16
    AF = mybir.ActivationFunctionType
    OP = mybir.AluOpType

    nc = bacc.Bacc("TRN2", target_bir_lowering=False, debug=False, num_devices=N_CORES)

    xt_q = nc.dram_tensor("xt_q", [D, S], F32, kind="ExternalInput")
    xt_k = nc.dram_tensor("xt_k", [D, S], F32, kind="ExternalInput")
    xt_v = nc.dram_tensor("xt_v", [D, S], F32, kind="ExternalInput")
    x_res = nc.dram_tensor("x_res", [512, D], F32, kind="ExternalInput")
    w_q = nc.dram_tensor("w_q", [D, 256], F32, kind="ExternalInput")
    w_k = nc.dram_tensor("w_k", [D, 256], F32, kind="ExternalInput")
    w_v = nc.dram_tensor("w_v", [D, 256], F32, kind="ExternalInput")
    wo_loc = nc.dram_tensor("wo_loc", [256, D], F32, kind="ExternalInput")
    gamma = nc.dram_tensor("gamma", [D], F32, kind="ExternalInput")
    beta = nc.dram_tensor("beta", [D], F32, kind="ExternalInput")
    causal = nc.dram_tensor("causal", [128, 128], F32, kind="ExternalInput")
    ident = nc.dram_tensor("ident", [128, 128], F32, kind="ExternalInput")
    if general:
        amask = nc.dram_tensor("amask", [S, S], F32, kind="ExternalInput")

    attn_p = nc.dram_tensor("attn_p", [4, S, S], F32, kind="ExternalOutput")
    out_p = nc.dram_tensor("out_p", [512, D], F32, kind="ExternalOutput")

    rs_in = nc.dram_tensor("rs_in", [S, D], F32)
    rs_out = nc.dram_tensor("rs_out", [512, D], F32)

    import bass_rust  # noqa: F401
    import concourse.bass as bass  # noqa: F401

    with tile.TileContext(nc) as tc:
        with tc.tile_pool(name="const", bufs=1) as const_pool, \
             tc.tile_pool(name="qtkt", bufs=1) as qt_pool, \
             tc.tile_pool(name="vp", bufs=1) as v_pool, \
             tc.tile_pool(name="small", bufs=8) as small, \
             tc.tile_pool(name="mm_ps", bufs=4, space="PSUM") as mm_ps, \
             tc.tile_pool(name="ctx_ps", bufs=2, space="PSUM") as ctx_ps_pool, \
             tc.tile_pool(name="pt_ps", bufs=2, space="PSUM") as pt_ps_pool:

            # ---- constants ----
            idt = const_pool.tile([128, 128], F32R)
            nc.sync.dma_start(out=idt, in_=ident[:].bitcast(F32R))
            causal_sb = const_pool.tile([128, 128], F32)
            nc.sync.dma_start(out=causal_sb, in_=causal[:])
            gamma_bc = const_pool.tile([128, D], F32)
            nc.gpsimd.dma_start(
                out=gamma_bc,
                in_=bass.AP(tensor=gamma[:].tensor, offset=0, ap=[[0, 128], [1, D]]),
            )
            beta_bc = const_pool.tile([128, D], F32)
            nc.gpsimd.dma_start(
                out=beta_bc,
                in_=bass.AP(tensor=beta[:].tensor, offset=0, ap=[[0, 128], [1, D]]),
            )
            eps_sb = const_pool.tile([128, 1], F32)
            nc.vector.memset(eps_sb, LN_EPS)

            # ---- phase 1: projections ----
            QT = {}
            KT = {}
            with tc.tile_pool(name="xt", bufs=10) as xt_pool, \
                 tc.tile_pool(name="wp", bufs=3) as w_pool:
                for tgt, xt_dram, w_dram in (("q", xt_q, w_q), ("k", xt_k, w_k)):
                    w_sb = w_pool.tile([128, 8, 256], F32R, tag="w", name=f"w_{tgt}")
                    for ci in range(8):
                        nc.sync.dma_start(
                            out=w_sb[:, ci, :],
                            in_=w_dram[128 * ci:128 * ci + 128, :].bitcast(F32R),
                        )
                    xch = []
                    for ci in range(8):
                        xc = xt_pool.tile([128, S], F32R, tag="xt", name=f"x_{tgt}{ci}")
                        nc.sync.dma_start(
                            out=xc, in_=xt_dram[128 * ci:128 * ci + 128, :].bitcast(F32R)
                        )
                        xch.append(xc)
                    for pair in range(2):
                        dstt = qt_pool.tile([128, S], F32R, name=f"{tgt}t_{pair}")
                        for q4 in range(4):
                            ps = mm_ps.tile([128, 512], F32, tag="mm", name="proj_ps")
                            for ci in range(8):
                                nc.tensor.matmul(
                                    ps,
                                    w_sb[:, ci, 128 * pair:128 * pair + 128],
                                    xch[ci][:, 512 * q4:512 * q4 + 512],
                                    start=(ci == 0),
                                    stop=(ci == 7),
                                )
                            if q4 % 2 == 0:
                                nc.vector.tensor_copy(dstt[:, 512 * q4:512 * q4 + 512], ps)
                            else:
                                nc.scalar.copy(dstt[:, 512 * q4:512 * q4 + 512], ps)
                        if tgt == "q":
                            QT[pair] = dstt
                        else:
                            KT[pair] = dstt

                # V in natural [s, d_v*4] layout
                wv_sb = w_pool.tile([128, 8, 256], F32R, tag="w", name="w_v_sb")
                for ci in range(8):
                    nc.sync.dma_start(
                        out=wv_sb[:, ci, :],
                        in_=w_v[128 * ci:128 * ci + 128, :].bitcast(F32R),
                    )
                xchv = []
                for ci in range(8):
                    xc = xt_pool.tile([128, S], F32R, tag="xt", name=f"x_v{ci}")
                    nc.sync.dma_start(
                        out=xc, in_=xt_v[128 * ci:128 * ci + 128, :].bitcast(F32R)
                    )
                    xchv.append(xc)
                v_sb = v_pool.tile([128, 16, 256], BF16)
                for st in range(16):
                    ps = mm_ps.tile([128, 256], F32, tag="mm", name="v_ps")
                    for ci in range(8):
                        nc.tensor.matmul(
                            ps,
                            xchv[ci][:, 128 * st:128 * st + 128],
                            wv_sb[:, ci, :],
                            start=(ci == 0),
                            stop=(ci == 7),
                        )
                    if st % 2 == 0:
                        nc.vector.tensor_copy(v_sb[:, st, :], ps)
                    else:
                        nc.scalar.copy(v_sb[:, st, :], ps)

            # ---- phase 2: attention ----
            with tc.tile_pool(name="ap", bufs=10) as a_pool, \
                 tc.tile_pool(name="ptp", bufs=3) as pt_pool, \
                 tc.tile_pool(name="ctxsb", bufs=1) as ctx_sb_pool, \
                 tc.tile_pool(name="wo", bufs=1) as wo_pool, \
                 tc.tile_pool(name="mk", bufs=3) as mk_pool:

                wo_sb = wo_pool.tile([128, 2, D], F32R)
                for pair in range(2):
                    nc.sync.dma_start(
                        out=wo_sb[:, pair, :],
                        in_=wo_loc[128 * pair:128 * pair + 128, :].bitcast(F32R),
                    )

                ctxT = {}
                for pair in range(2):
                    ctxT[pair] = ctx_sb_pool.tile([128, S], F32R, name=f"ctxT_{pair}")

                cpy = [0]

                def pcopy(dst, src):
                    # alternate PSUM->SBUF copies between DVE and ACT
                    if cpy[0] % 2 == 0:
                        nc.vector.tensor_copy(dst, src)
                    else:
                        nc.scalar.copy(dst, src)
                    cpy[0] += 1

                for pair in range(2):
                    for qs in range(4):
                        A = {}
                        # --- scores + softmax for the 4 q-subtiles ---
                        for qi in range(4):
                            qstart = 512 * qs + 128 * qi
                            klen = S if general else qstart + 128
                            nch = (klen + 511) // 512
                            sums = {}
                            for h in range(2):
                                A[(h, qi)] = a_pool.tile(
                                    [128, 2048], F32R, tag="a", name=f"a{pair}{qs}{qi}{h}"
                                )
                                sums[h] = small.tile(
                                    [128, 8], F32, tag="sums", name=f"s{pair}{qs}{qi}{h}"
                                )
                            for j in range(nch):
                                c0 = 512 * j
                                w = min(512, klen - c0)
                                pss = {}
                                for h in range(2):
                                    ps = mm_ps.tile(
                                        [128, 512], F32, tag="mm", name=f"sc{h}"
                                    )
                                    pb = 64 * h
                                    nc.tensor.matmul(
                                        ps[:, 0:w],
                                        QT[pair][pb:pb + 64, qstart:qstart + 128],
                                        KT[pair][pb:pb + 64, c0:c0 + w],
                                        start=True,
                                        stop=True,
                                    )
                                    pss[h] = ps
                                if general:
                                    mk = mk_pool.tile([128, 512], F32, tag="mk", name="mk")
                                    nc.sync.dma_start(
                                        out=mk[:, 0:w],
                                        in_=amask[qstart:qstart + 128, c0:c0 + w],
                                    )
                                for h in range(2):
                                    ps = pss[h]
                                    if general:
                                        nc.vector.tensor_add(ps[:, 0:w], ps[:, 0:w], mk[:, 0:w])
                                    elif j == nch - 1:
                                        nc.vector.tensor_add(
                                            ps[:, w - 128:w], ps[:, w - 128:w], causal_sb
                                        )
                                    nc.scalar.activation(
                                        A[(h, qi)][:, c0:c0 + w],
                                        ps[:, 0:w],
                                        AF.Exp,
                                        scale=0.125,
                                        accum_out=sums[h][:, j:j + 1],
                                    )
                            # normalize + write attn rows
                            for h in range(2):
                                s1 = small.tile([128, 1], F32, tag="s1", name="s1")
                                if nch > 1:
                                    nc.vector.reduce_sum(
                                        s1, sums[h][:, 0:nch], axis=mybir.AxisListType.X
                                    )
                                else:
                                    nc.vector.tensor_copy(s1, sums[h][:, 0:1])
                                rr = small.tile([128, 1], F32, tag="rr", name="rr")
                                nc.vector.reciprocal(rr, s1)
                                at = A[(h, qi)]
                                nc.vector.tensor_scalar_mul(
                                    at[:, 0:klen], at[:, 0:klen], rr
                                )
                                h_loc = 2 * pair + h
                                nc.sync.dma_start(
                                    out=attn_p[h_loc, qstart:qstart + 128, 0:klen],
                                    in_=at[:, 0:klen].bitcast(F32),
                                )

                        # --- context: transpose P, col-tiled matmuls ---
                        ctx = ctx_ps_pool.tile([128, 512], F32, tag="ctx", name="ctx")
                        nkb = 16 if general else 4 * qs + 4
                        for kb in range(nkb):
                            jstart = 0 if general else max(0, kb - 4 * qs)
                            cw = 512 - 128 * jstart
                            for h in range(2):
                                ptp = pt_ps_pool.tile(
                                    [128, 512], F32R, tag="pt", name="ptps"
                                )
                                for qi in range(jstart, 4):
                                    nc.tensor.transpose(
                                        ptp[:, 128 * qi:128 * qi + 128],
                                        A[(h, qi)][:, 128 * kb:128 * kb + 128],
                                        idt,
                                    )
                                pts = pt_pool.tile([128, 512], BF16, tag="pts", name="pts")
                                pcopy(pts[:, 128 * jstart:512], ptp[:, 128 * jstart:512])
                                nc.tensor.matmul(
                                    ctx[64 * h:64 * h + 64, 128 * jstart:512],
                                    v_sb[:, kb, 128 * pair + 64 * h:128 * pair + 64 * h + 64],
                                    pts[:, 128 * jstart:512],
                                    start=(kb == 0),
                                    stop=(kb == nkb - 1),
                                )
                        pcopy(ctxT[pair][:, 512 * qs:512 * qs + 512], ctx)

                # ---- fc partials (row-parallel) ----
                for qt in range(16):
                    for nt in range(2):
                        ps = mm_ps.tile([128, 512], F32, tag="mm", name="fc_ps")
                        for pair in range(2):
                            nc.tensor.matmul(
                                ps,
                                ctxT[pair][:, 128 * qt:128 * qt + 128],
                                wo_sb[:, pair, 512 * nt:512 * nt + 512],
                                start=(pair == 0),
                                stop=(pair == 1),
                            )
                        fco = pt_pool.tile([128, 512], F32, tag="fco", name="fco")
                        pcopy(fco, ps)
                        nc.sync.dma_start(
                            out=rs_in[128 * qt:128 * qt + 128, 512 * nt:512 * nt + 512],
                            in_=fco,
                        )

            # ---- phase 3: reduce-scatter over the 4-core batch group ----
            nc.gpsimd.collective_compute(
                "ReduceScatter",
                OP.add,
                replica_groups=[[0, 1, 2, 3], [4, 5, 6, 7]],
                ins=[rs_in[:]],
                outs=[rs_out[:]],
            )

            # ---- phase 4: residual + layernorm on this core's quarter ----
            with tc.tile_pool(name="ln", bufs=2) as ln_pool:
                for qt in range(4):
                    fc_sb = ln_pool.tile([128, D], F32, tag="fc", name="fc_sb")
                    nc.sync.dma_start(
                        out=fc_sb, in_=rs_out[128 * qt:128 * qt + 128, :]
                    )
                    x_sb = ln_pool.tile([128, D], F32, tag="x", name="x_sb")
                    nc.sync.dma_start(out=x_sb, in_=x_res[128 * qt:128 * qt + 128, :])
                    t = ln_pool.tile([128, D], F32, tag="t", name="t_sb")
                    nc.vector.tensor_add(t, fc_sb, x_sb)
                    st = small.tile([128, 2, 6], F32, tag="bnst", name="bnst")
                    for sub in range(2):
                        nc.vector.bn_stats(st[:, sub, :], t[:, 512 * sub:512 * sub + 512])
                    mv = small.tile([128, 2], F32, tag="mv", name="mv")
                    nc.vector.bn_aggr(mv, st)
                    sd = small.tile([128, 1], F32, tag="sd", name="sd")
                    nc.scalar.activation(
                        sd, mv[:, 1:2], AF.Sqrt, bias=eps_sb, scale=1.0
                    )
                    rstd = small.tile([128, 1], F32, tag="rstd", name="rstd")
                    nc.vector.reciprocal(rstd, sd)
                    nc.vector.tensor_scalar(
                        t, t, mv[:, 0:1], rstd, OP.subtract, OP.mult
                    )
                    nc.vector.tensor_mul(t, t, gamma_bc)
                    nc.vector.tensor_add(t, t, beta_bc)
                    nc.sync.dma_start(
                        out=out_p[128 * qt:128 * qt + 128, :], in_=t
                    )

    nc.compile()
    return nc


def _get_nc(general: bool):
    key = ("gen" if general else "causal")
    if key not in _cache:
        _cache[key] = _build(general)
    return _cache[key]


def kernel(input_Q, input_K, input_V, attn_mask, W_Q, W_K, W_V, W_O, ln_gamma, ln_beta):
    from concourse.bass_utils import run_bass_kernel_spmd

    input_Q = np.asarray(input_Q, dtype=np.float32)
    input_K = np.asarray(input_K, dtype=np.float32)
    input_V = np.asarray(input_V, dtype=np.float32)
    attn_mask = np.asarray(attn_mask)
    W_Q = np.asarray(W_Q, dtype=np.float32)
    W_K = np.asarray(W_K, dtype=np.float32)
    W_V = np.asarray(W_V, dtype=np.float32)
    W_O = np.asarray(W_O, dtype=np.float32)
    ln_gamma = np.asarray(ln_gamma, dtype=np.float32)
    ln_beta = np.asarray(ln_beta, dtype=np.float32)

    causal_ref = np.triu(np.ones((S, S), dtype=bool), k=1)
    general = not all(np.array_equal(attn_mask[b], causal_ref) for b in range(B))

    nc = _get_nc(general)

    tri = np.triu(np.full((128, 128), NEG, dtype=np.float32), k=1)
    eye = np.eye(128, dtype=np.float32)

    in_maps = []
    for c in range(N_CORES):
        b, hg = divmod(c, 4)
        m = {
            "xt_q": np.ascontiguousarray(input_Q[b].T),
            "xt_k": np.ascontiguousarray(input_K[b].T),
            "xt_v": np.ascontiguousarray(input_V[b].T),
            "x_res": np.ascontiguousarray(input_Q[b, 512 * hg:512 * hg + 512, :]),
            "w_q": np.ascontiguousarray(W_Q[:, 256 * hg:256 * hg + 256]),
            "w_k": np.ascontiguousarray(W_K[:, 256 * hg:256 * hg + 256]),
            "w_v": np.ascontiguousarray(W_V[:, 256 * hg:256 * hg + 256]),
            "wo_loc": np.ascontiguousarray(W_O[256 * hg:256 * hg + 256, :]),
            "gamma": ln_gamma,
            "beta": ln_beta,
            "causal": tri,
            "ident": eye,
        }
        if general:
            m["amask"] = np.where(attn_mask[b], np.float32(NEG), np.float32(0.0))
        in_maps.append(m)

    res = run_bass_kernel_spmd(nc, in_maps, core_ids=list(range(N_CORES)))
    kernel.last_results = res

    out = np.empty((B, S, D), dtype=np.float32)
    attn = np.empty((B, H, S, S), dtype=np.float32)
    for c in range(N_CORES):
        b, hg = divmod(c, 4)
        attn[b, 4 * hg:4 * hg + 4] = res.results[c]["attn_p"]
        out[b, 512 * hg:512 * hg + 512, :] = res.results[c]["out_p"]
    return out, attn


# revision 6
# speedup vs baseline: 464.5649x; 464.5649x over previous
import sys

sys.path.insert(0, "/opt/trn_rl_repo")

import numpy as np

B, S, D, H, DK, DV = 2, 2048, 1024, 16, 64, 64
N_CORES = 8
LN_EPS = 1e-5
NEG = -1.0e9

_cache = {}


def _build(general: bool):
    import concourse.bacc as bacc
    import concourse.tile as tile
    from concourse import mybir

    F32 = mybir.dt.float32
    F32R = mybir.dt.float32r
    BF16 = mybir.dt.bfloat16
    AF = mybir.ActivationFunctionType
    OP = mybir.AluOpType

    nc = bacc.Bacc("TRN2", target_bir_lowering=False, debug=False, num_devices=N_CORES)

    xt_q = nc.dram_tensor("xt_q", [D, S], F32, kind="ExternalInput")
    xt_k = nc.dram_tensor("xt_k", [D, S], F32, kind="ExternalInput")
    xt_v = nc.dram_tensor("xt_v", [D, S], F32, kind="ExternalInput")
    x_res = nc.dram_tensor("x_res", [512, D], F32, kind="ExternalInput")
    w_q = nc.dram_tensor("w_q", [D, 256], F32, kind="ExternalInput")
    w_k = nc.dram_tensor("w_k", [D, 256], F32, kind="ExternalInput")
    w_v = nc.dram_tensor("w_v", [D, 256], F32, kind="ExternalInput")
    wo_loc = nc.dram_tensor("wo_loc", [256, D], F32, kind="ExternalInput")
    gamma = nc.dram_tensor("gamma", [D], F32, kind="ExternalInput")
    beta = nc.dram_tensor("beta", [D], F32, kind="ExternalInput")
    causal = nc.dram_tensor("causal", [128, 128], F32, kind="ExternalInput")
    ident = nc.dram_tensor("ident", [128, 128], F32, kind="ExternalInput")
    if general:
        amask = nc.dram_tensor("amask", [S, S], F32, kind="ExternalInput")

    attn_p = nc.dram_tensor("attn_p", [4, S, S], F32, kind="ExternalOutput")
    out_p = nc.dram_tensor("out_p", [512, D], F32, kind="ExternalOutput")

    rs_in = nc.dram_tensor("rs_in", [S, D], F32)
    rs_out = nc.dram_tensor("rs_out", [512, D], F32)

    import bass_rust  # noqa: F401
    import concourse.bass as bass  # noqa: F401

    with tile.TileContext(nc) as tc:
        with tc.tile_pool(name="const", bufs=1) as const_pool, \
             tc.tile_pool(name="qtkt", bufs=1) as qt_pool, \
             tc.tile_pool(name="vp", bufs=1) as v_pool, \
             tc.tile_pool(name="small", bufs=8) as small, \
             tc.tile_pool(name="mm_ps", bufs=4, space="PSUM") as mm_ps, \
             tc.tile_pool(name="ctx_ps", bufs=2, space="PSUM") as ctx_ps_pool, \
             tc.tile_pool(name="pt_ps", bufs=2, space="PSUM") as pt_ps_pool:

            # ---- constants ----
            idt = const_pool.tile([128, 128], F32R)
            nc.sync.dma_start(out=idt, in_=ident[:].bitcast(F32R))
            causal_sb = const_pool.tile([128, 128], F32)
            nc.sync.dma_start(out=causal_sb, in_=causal[:])
            gamma_bc = const_pool.tile([128, D], F32)
            nc.gpsimd.dma_start(
                out=gamma_bc,
                in_=bass.AP(tensor=gamma[:].tensor, offset=0, ap=[[0, 128], [1, D]]),
            )
            beta_bc = const_pool.tile([128, D], F32)
            nc.gpsimd.dma_start(
                out=beta_bc,
                in_=bass.AP(tensor=beta[:].tensor, offset=0, ap=[[0, 128], [1, D]]),
            )
            eps_sb = const_pool.tile([128, 1], F32)
            nc.vector.memset(eps_sb, LN_EPS)

            # ---- phase 1: projections ----
            QT = {}
            KT = {}
            with tc.tile_pool(name="xt", bufs=10) as xt_pool, \
                 tc.tile_pool(name="wp", bufs=3) as w_pool:
                for tgt, xt_dram, w_dram in (("q", xt_q, w_q), ("k", xt_k, w_k)):
                    w_sb = w_pool.tile([128, 8, 256], F32R, tag="w", name=f"w_{tgt}")
                    for ci in range(8):
                        nc.sync.dma_start(
                            out=w_sb[:, ci, :],
                            in_=w_dram[128 * ci:128 * ci + 128, :].bitcast(F32R),
                        )
                    xch = []
                    for ci in range(8):
                        xc = xt_pool.tile([128, S], F32R, tag="xt", name=f"x_{tgt}{ci}")
                        nc.sync.dma_start(
                            out=xc, in_=xt_dram[128 * ci:128 * ci + 128, :].bitcast(F32R)
                        )
                        xch.append(xc)
                    for pair in range(2):
                        dstt = qt_pool.tile([128, S], F32R, name=f"{tgt}t_{pair}")
                        for q4 in range(4):
                            ps = mm_ps.tile([128, 512], F32, tag="mm", name="proj_ps")
                            for ci in range(8):
                                nc.tensor.matmul(
                                    ps,
                                    w_sb[:, ci, 128 * pair:128 * pair + 128],
                                    xch[ci][:, 512 * q4:512 * q4 + 512],
                                    start=(ci == 0),
                                    stop=(ci == 7),
                                )
                            if q4 % 2 == 0:
                                nc.vector.tensor_copy(dstt[:, 512 * q4:512 * q4 + 512], ps)
                            else:
                                nc.scalar.copy(dstt[:, 512 * q4:512 * q4 + 512], ps)
                        if tgt == "q":
                            QT[pair] = dstt
                        else:
                            KT[pair] = dstt

                # V in natural [s, d_v*4] layout
                wv_sb = w_pool.tile([128, 8, 256], F32R, tag="w", name="w_v_sb")
                for ci in range(8):
                    nc.sync.dma_start(
                        out=wv_sb[:, ci, :],
                        in_=w_v[128 * ci:128 * ci + 128, :].bitcast(F32R),
                    )
                xchv = []
                for ci in range(8):
                    xc = xt_pool.tile([128, S], F32R, tag="xt", name=f"x_v{ci}")
                    nc.sync.dma_start(
                        out=xc, in_=xt_v[128 * ci:128 * ci + 128, :].bitcast(F32R)
                    )
                    xchv.append(xc)
                v_sb = v_pool.tile([128, 16, 256], BF16)
                for st in range(16):
                    ps = mm_ps.tile([128, 256], F32, tag="mm", name="v_ps")
                    for ci in range(8):
                        nc.tensor.matmul(
                            ps,
                            xchv[ci][:, 128 * st:128 * st + 128],
                            wv_sb[:, ci, :],
                            start=(ci == 0),
                            stop=(ci == 7),
                        )
                    if st % 2 == 0:
                        nc.vector.tensor_copy(v_sb[:, st, :], ps)
                    else:
                        nc.scalar.copy(v_sb[:, st, :], ps)

            # ---- phase 2: attention ----
            with tc.tile_pool(name="ap", bufs=10) as a_pool, \
                 tc.tile_pool(name="ptp", bufs=3) as pt_pool, \
                 tc.tile_pool(name="ctxsb", bufs=1) as ctx_sb_pool, \
                 tc.tile_pool(name="wo", bufs=1) as wo_pool, \
                 tc.tile_pool(name="mk", bufs=3) as mk_pool:

                wo_sb = wo_pool.tile([128, 2, D], F32R)
                for pair in range(2):
                    nc.sync.dma_start(
                        out=wo_sb[:, pair, :],
                        in_=wo_loc[128 * pair:128 * pair + 128, :].bitcast(F32R),
                    )

                ctxT = {}
                for pair in range(2):
                    ctxT[pair] = ctx_sb_pool.tile([128, S], F32R, name=f"ctxT_{pair}")

                cpy = [0]

                def pcopy(dst, src):
                    # alternate PSUM->SBUF copies between DVE and ACT
                    if cpy[0] % 2 == 0:
                        nc.vector.tensor_copy(dst, src)
                    else:
                        nc.scalar.copy(dst, src)
                    cpy[0] += 1

                for pair in range(2):
                    for qs in range(4):
                        A = {}
                        # --- scores + softmax for the 4 q-subtiles ---
                        for qi in range(4):
                            qstart = 512 * qs + 128 * qi
                            klen = S if general else qstart + 128
                            nch = (klen + 511) // 512
                            sums = {}
                            for h in range(2):
                                A[(h, qi)] = a_pool.tile(
                                    [128, 2048], F32R, tag="a", name=f"a{pair}{qs}{qi}{h}"
                                )
                                sums[h] = small.tile(
                                    [128, 8], F32, tag="sums", name=f"s{pair}{qs}{qi}{h}"
                                )
                            for j in range(nch):
                                c0 = 512 * j
                                w = min(512, klen - c0)
                                pss = {}
                                for h in range(2):
                                    ps = mm_ps.tile(
                                        [128, 512], F32, tag="mm", name=f"sc{h}"
                                    )
                                    pb = 64 * h
                                    nc.tensor.matmul(
                                        ps[:, 0:w],
                                        QT[pair][pb:pb + 64, qstart:qstart + 128],
                                        KT[pair][pb:pb + 64, c0:c0 + w],
                                        start=True,
                                        stop=True,
                                    )
                                    pss[h] = ps
                                if general:
                                    mk = mk_pool.tile([128, 512], F32, tag="mk", name="mk")
                                    nc.sync.dma_start(
                                        out=mk[:, 0:w],
                                        in_=amask[qstart:qstart + 128, c0:c0 + w],
                                    )
                                for h in range(2):
                                    ps = pss[h]
                                    if general:
                                        nc.vector.tensor_add(ps[:, 0:w], ps[:, 0:w], mk[:, 0:w])
                                    elif j == nch - 1:
                                        nc.vector.tensor_add(
                                            ps[:, w - 128:w], ps[:, w - 128:w], causal_sb
                                        )
                                    nc.scalar.activation(
                                        A[(h, qi)][:, c0:c0 + w],
                                        ps[:, 0:w],
                                        AF.Exp,
                                        scale=0.125,
                                        accum_out=sums[h][:, j:j + 1],
                                    )
                            # normalize + write attn rows
                            for h in range(2):
                                s1 = small.tile([128, 1], F32, tag="s1", name="s1")
                                if nch > 1:
                                    nc.vector.reduce_sum(
                                        s1, sums[h][:, 0:nch], axis=mybir.AxisListType.X
                                    )
                                else:
                                    nc.vector.tensor_copy(s1, sums[h][:, 0:1])
                                rr = small.tile([128, 1], F32, tag="rr", name="rr")
                                nc.vector.reciprocal(rr, s1)
                                at = A[(h, qi)]
                                nc.vector.tensor_scalar_mul(
                                    at[:, 0:klen], at[:, 0:klen], rr
                                )
                                h_loc = 2 * pair + h
                                nc.sync.dma_start(
                                    out=attn_p[h_loc, qstart:qstart + 128, 0:klen],
                                    in_=at[:, 0:klen].bitcast(F32),
                                )

                        # --- context: transpose P, col-tiled matmuls ---
                        ctx = ctx_ps_pool.tile([128, 512], F32, tag="ctx", name="ctx")
                        nkb = 16 if general else 4 * qs + 4
                        for kb in range(nkb):
                            jstart = 0 if general else max(0, kb - 4 * qs)
                            cw = 512 - 128 * jstart
                            for h in range(2):
                                ptp = pt_ps_pool.tile(
                                    [128, 512], F32R, tag="pt", name="ptps"
                                )
                                for qi in range(jstart, 4):
                                    nc.tensor.transpose(
                                        ptp[:, 128 * qi:128 * qi + 128],
                                        A[(h, qi)][:, 128 * kb:128 * kb + 128],
                                        idt,
                                    )
                                pts = pt_pool.tile([128, 512], BF16, tag="pts", name="pts")
                                pcopy(pts[:, 128 * jstart:512], ptp[:, 128 * jstart:512])
                                nc.tensor.matmul(
                                    ctx[64 * h:64 * h + 64, 128 * jstart:512],
                                    v_sb[:, kb, 128 * pair + 64 * h:128 * pair + 64 * h + 64],
                                    pts[:, 128 * jstart:512],
                                    start=(kb == 0),
                                    stop=(kb == nkb - 1),
                                )
                        pcopy(ctxT[pair][:, 512 * qs:512 * qs + 512], ctx)

                # ---- fc partials (row-parallel) ----
                for qt in range(16):
                    for nt in range(2):
                        ps = mm_ps.tile([128, 512], F32, tag="mm", name="fc_ps")
                        for pair in range(2):
                            nc.tensor.matmul(
                                ps,
                                ctxT[pair][:, 128 * qt:128 * qt + 128],
                                wo_sb[:, pair, 512 * nt:512 * nt + 512],
                                start=(pair == 0),
                                stop=(pair == 1),
                            )
                        fco = pt_pool.tile([128, 512], F32, tag="fco", name="fco")
                        pcopy(fco, ps)
                        nc.sync.dma_start(
                            out=rs_in[128 * qt:128 * qt + 128, 512 * nt:512 * nt + 512],
                            in_=fco,
                        )

            # ---- phase 3: reduce-scatter over the 4-core batch group ----
            nc.gpsimd.collective_compute(
                "ReduceScatter",
                OP.add,
                replica_groups=[[0, 1, 2, 3], [4, 5, 6, 7]],
                ins=[rs_in[:]],
                outs=[rs_out[:]],
            )

            # ---- phase 4: residual + layernorm on this core's quarter ----
            with tc.tile_pool(name="ln", bufs=2) as ln_pool:
                for qt in range(4):
                    fc_sb = ln_pool.tile([128, D], F32, tag="fc", name="fc_sb")
                    nc.sync.dma_start(
                        out=fc_sb, in_=rs_out[128 * qt:128 * qt + 128, :]
                    )
                    x_sb = ln_pool.tile([128, D], F32, tag="x", name="x_sb")
                    nc.sync.dma_start(out=x_sb, in_=x_res[128 * qt:128 * qt + 128, :])
                    t = ln_pool.tile([128, D], F32, tag="t", name="t_sb")
                    nc.vector.tensor_add(t, fc_sb, x_sb)
                    st = small.tile([128, 2, 6], F32, tag="bnst", name="bnst")
                    for sub in range(2):
                        nc.vector.bn_stats(st[:, sub, :], t[:, 512 * sub:512 * sub + 512])
                    mv = small.tile([128, 2], F32, tag="mv", name="mv")
                    nc.vector.bn_aggr(mv, st)
                    sd = small.tile([128, 1], F32, tag="sd", name="sd")
                    nc.scalar.activation(
                        sd, mv[:, 1:2], AF.Sqrt, bias=eps_sb, scale=1.0
                    )
                    rstd = small.tile([128, 1], F32, tag="rstd", name="rstd")
                    nc.vector.reciprocal(rstd, sd)
                    nc.vector.tensor_scalar(
                        t, t, mv[:, 0:1], rstd, OP.subtract, OP.mult
                    )
                    nc.vector.tensor_mul(t, t, gamma_bc)
                    nc.vector.tensor_add(t, t, beta_bc)
                    nc.sync.dma_start(
                        out=out_p[128 * qt:128 * qt + 128, :], in_=t
                    )

    nc.compile()
    return nc


def _get_nc(general: bool):
    key = ("gen" if general else "causal")
    if key not in _cache:
        _cache[key] = _build(general)
    return _cache[key]


def kernel(input_Q, input_K, input_V, attn_mask, W_Q, W_K, W_V, W_O, ln_gamma, ln_beta):
    from concourse.bass_utils import run_bass_kernel_spmd

    input_Q = np.asarray(input_Q, dtype=np.float32)
    input_K = np.asarray(input_K, dtype=np.float32)
    input_V = np.asarray(input_V, dtype=np.float32)
    attn_mask = np.asarray(attn_mask)
    W_Q = np.asarray(W_Q, dtype=np.float32)
    W_K = np.asarray(W_K, dtype=np.float32)
    W_V = np.asarray(W_V, dtype=np.float32)
    W_O = np.asarray(W_O, dtype=np.float32)
    ln_gamma = np.asarray(ln_gamma, dtype=np.float32)
    ln_beta = np.asarray(ln_beta, dtype=np.float32)

    causal_ref = np.triu(np.ones((S, S), dtype=bool), k=1)
    general = not all(np.array_equal(attn_mask[b], causal_ref) for b in range(B))

    nc = _get_nc(general)

    tri = np.triu(np.full((128, 128), NEG, dtype=np.float32), k=1)
    eye = np.eye(128, dtype=np.float32)

    in_maps = []
    for c in range(N_CORES):
        b, hg = divmod(c, 4)
        m = {
            "xt_q": np.ascontiguousarray(input_Q[b].T),
            "xt_k": np.ascontiguousarray(input_K[b].T),
            "xt_v": np.ascontiguousarray(input_V[b].T),
            "x_res": np.ascontiguousarray(input_Q[b, 512 * hg:512 * hg + 512, :]),
            "w_q": np.ascontiguousarray(W_Q[:, 256 * hg:256 * hg + 256]),
            "w_k": np.ascontiguousarray(W_K[:, 256 * hg:256 * hg + 256]),
            "w_v": np.ascontiguousarray(W_V[:, 256 * hg:256 * hg + 256]),
            "wo_loc": np.ascontiguousarray(W_O[256 * hg:256 * hg + 256, :]),
            "gamma": ln_gamma,
            "beta": ln_beta,
            "causal": tri,
            "ident": eye,
        }
        if general:
            m["amask"] = np.where(attn_mask[b], np.float32(NEG), np.float32(0.0))
        in_maps.append(m)

    import os
    trace = bool(os.environ.get("KERNEL_TRACE"))
    kw = {}
    if trace:
        kw = dict(trace=True, tmpdir=os.environ.get("KERNEL_TRACE_DIR") or None)
    res = run_bass_kernel_spmd(nc, in_maps, core_ids=list(range(N_CORES)), **kw)
    kernel.last_results = res
    kernel.last_in_maps = in_maps

    out = np.empty((B, S, D), dtype=np.float32)
    attn = np.empty((B, H, S, S), dtype=np.float32)
    for c in range(N_CORES):
        b, hg = divmod(c, 4)
        attn[b, 4 * hg:4 * hg + 4] = res.results[c]["attn_p"]
        out[b, 512 * hg:512 * hg + 512, :] = res.results[c]["out_p"]
    return out, attn


# revision 9
# speedup vs baseline: 24417.8131x; 52.5606x over previous
import sys

sys.path.insert(0, "/opt/trn_rl_repo")

import numpy as np

B, S, D, H, DK, DV = 2, 2048, 1024, 16, 64, 64
N_CORES = 8
LN_EPS = 1e-5
NEG = -1.0e9

_cache = {}


def _build(general: bool, single: bool = False, ablate: str = ""):
    import concourse.bacc as bacc
    import concourse.bass as bass
    import concourse.tile as tile
    from concourse import mybir

    F32 = mybir.dt.float32
    F32R = mybir.dt.float32r
    F16 = mybir.dt.float16
    AF = mybir.ActivationFunctionType
    OP = mybir.AluOpType

    nc = bacc.Bacc("TRN2", target_bir_lowering=False, debug=False,
                   num_devices=(1 if single else N_CORES))

    # fp16 inputs for all matmul operands (10-bit mantissa ~= tf32 precision)
    xt_q = nc.dram_tensor("xt_q", [D, S], F16, kind="ExternalInput")
    xt_k = nc.dram_tensor("xt_k", [D, S], F16, kind="ExternalInput")
    xt_v = nc.dram_tensor("xt_v", [D, S], F16, kind="ExternalInput")
    x_res = nc.dram_tensor("x_res", [512, D], F32, kind="ExternalInput")
    w_q = nc.dram_tensor("w_q", [D, 256], F16, kind="ExternalInput")
    w_k = nc.dram_tensor("w_k", [D, 256], F16, kind="ExternalInput")
    w_v = nc.dram_tensor("w_v", [D, 256], F16, kind="ExternalInput")
    wo_loc = nc.dram_tensor("wo_loc", [256, D], F16, kind="ExternalInput")
    gamma = nc.dram_tensor("gamma", [D], F32, kind="ExternalInput")
    beta = nc.dram_tensor("beta", [D], F32, kind="ExternalInput")
    causal = nc.dram_tensor("causal", [128, 128], F32, kind="ExternalInput")
    ident = nc.dram_tensor("ident", [128, 128], F32, kind="ExternalInput")
    if general:
        amask = nc.dram_tensor("amask", [S, S], F32, kind="ExternalInput")

    attn_p = nc.dram_tensor("attn_p", [4, S, S], F32, kind="ExternalOutput")
    out_p = nc.dram_tensor("out_p", [512, D], F32, kind="ExternalOutput")

    rs_in = nc.dram_tensor("rs_in", [S, D], F32)
    rs_out = nc.dram_tensor("rs_out", [512, D], F32)

    with tile.TileContext(nc) as tc:
        with tc.tile_pool(name="const", bufs=1) as const_pool, \
             tc.tile_pool(name="qtkt", bufs=1) as qt_pool, \
             tc.tile_pool(name="vp", bufs=1) as v_pool, \
             tc.tile_pool(name="small", bufs=8) as small, \
             tc.tile_pool(name="mm_ps", bufs=4, space="PSUM") as mm_ps, \
             tc.tile_pool(name="ctx_ps", bufs=2, space="PSUM") as ctx_ps_pool, \
             tc.tile_pool(name="pt_ps", bufs=2, space="PSUM") as pt_ps_pool:

            # ---- constants ----
            idt = const_pool.tile([128, 128], F32R)
            nc.sync.dma_start(out=idt, in_=ident[:].bitcast(F32R))
            causal_sb = const_pool.tile([128, 128], F32)
            nc.sync.dma_start(out=causal_sb, in_=causal[:])
            gamma_bc = const_pool.tile([128, D], F32)
            nc.gpsimd.dma_start(
                out=gamma_bc,
                in_=bass.AP(tensor=gamma[:].tensor, offset=0, ap=[[0, 128], [1, D]]),
            )
            beta_bc = const_pool.tile([128, D], F32)
            nc.gpsimd.dma_start(
                out=beta_bc,
                in_=bass.AP(tensor=beta[:].tensor, offset=0, ap=[[0, 128], [1, D]]),
            )
            eps_sb = const_pool.tile([128, 1], F32)
            nc.vector.memset(eps_sb, LN_EPS)

            # ---- phase 1: projections ----
            QT = {}
            KT = {}
            with tc.tile_pool(name="xt", bufs=10) as xt_pool, \
                 tc.tile_pool(name="wp", bufs=3) as w_pool:
                for tgt, xt_dram, w_dram in (("q", xt_q, w_q), ("k", xt_k, w_k)):
                    w_sb = w_pool.tile([128, 8, 256], F16, tag="w", name=f"w_{tgt}")
                    for ci in range(8):
                        nc.sync.dma_start(
                            out=w_sb[:, ci, :],
                            in_=w_dram[128 * ci:128 * ci + 128, :],
                        )
                    xch = []
                    for ci in range(8):
                        xc = xt_pool.tile([128, S], F16, tag="xt", name=f"x_{tgt}{ci}")
                        nc.sync.dma_start(
                            out=xc, in_=xt_dram[128 * ci:128 * ci + 128, :]
                        )
                        xch.append(xc)
                    for pair in range(2):
                        dstt = qt_pool.tile([128, S], F16, name=f"{tgt}t_{pair}")
                        for q4 in range(4):
                            ps = mm_ps.tile([128, 512], F32, tag="mm", name="proj_ps")
                            for ci in range(8):
                                nc.tensor.matmul(
                                    ps,
                                    w_sb[:, ci, 128 * pair:128 * pair + 128],
                                    xch[ci][:, 512 * q4:512 * q4 + 512],
                                    start=(ci == 0),
                                    stop=(ci == 7),
                                )
                            if q4 % 2 == 0:
                                nc.vector.tensor_copy(dstt[:, 512 * q4:512 * q4 + 512], ps)
                            else:
                                nc.scalar.copy(dstt[:, 512 * q4:512 * q4 + 512], ps)
                        if tgt == "q":
                            QT[pair] = dstt
                        else:
                            KT[pair] = dstt

                # V in natural [s, d_v*4] layout
                wv_sb = w_pool.tile([128, 8, 256], F16, tag="w", name="w_v_sb")
                for ci in range(8):
                    nc.sync.dma_start(
                        out=wv_sb[:, ci, :],
                        in_=w_v[128 * ci:128 * ci + 128, :],
                    )
                xchv = []
                for ci in range(8):
                    xc = xt_pool.tile([128, S], F16, tag="xt", name=f"x_v{ci}")
                    nc.sync.dma_start(
                        out=xc, in_=xt_v[128 * ci:128 * ci + 128, :]
                    )
                    xchv.append(xc)
                v_sb = v_pool.tile([128, 16, 256], F16)
                for st in range(16):
                    ps = mm_ps.tile([128, 256], F32, tag="mm", name="v_ps")
                    for ci in range(8):
                        nc.tensor.matmul(
                            ps,
                            xchv[ci][:, 128 * st:128 * st + 128],
                            wv_sb[:, ci, :],
                            start=(ci == 0),
                            stop=(ci == 7),
                        )
                    if st % 2 == 0:
                        nc.vector.tensor_copy(v_sb[:, st, :], ps)
                    else:
                        nc.scalar.copy(v_sb[:, st, :], ps)

            # ---- phase 2: attention ----
            with tc.tile_pool(name="ap", bufs=12) as a_pool, \
                 tc.tile_pool(name="ptp", bufs=4) as pt_pool, \
                 tc.tile_pool(name="ctxsb", bufs=1) as ctx_sb_pool, \
                 tc.tile_pool(name="wo", bufs=1) as wo_pool, \
                 tc.tile_pool(name="mk", bufs=3) as mk_pool:

                wo_sb = wo_pool.tile([128, 2, D], F16)
                for pair in range(2):
                    nc.sync.dma_start(
                        out=wo_sb[:, pair, :],
                        in_=wo_loc[128 * pair:128 * pair + 128, :],
                    )

                ctxT = {}
                for pair in range(2):
                    ctxT[pair] = ctx_sb_pool.tile([128, S], F16, name=f"ctxT_{pair}")

                cpy = [0]

                def pcopy(dst, src):
                    # PSUM->SBUF copies: 3/5 on DVE, 2/5 on ACT (ACT is busier)
                    if cpy[0] % 5 in (0, 2, 4):
                        nc.vector.tensor_copy(dst, src)
                    else:
                        nc.scalar.copy(dst, src)
                    cpy[0] += 1

                for pair in range(0 if ablate == "proj" else 2):
                    for qs in range(4):
                        A = {}
                        # --- scores + softmax for the 4 q-subtiles ---
                        for qi in range(4):
                            qstart = 512 * qs + 128 * qi
                            klen = S if general else qstart + 128
                            nch = (klen + 511) // 512
                            A[(0, qi)] = a_pool.tile(
                                [128, 2048], F32R, tag="a", name=f"a{pair}{qs}{qi}0"
                            )
                            A[(1, qi)] = a_pool.tile(
                                [128, 2048], F32R, tag="a", name=f"a{pair}{qs}{qi}1"
                            )
                            sums = small.tile(
                                [128, 2, 4], F32, tag="sums", name=f"s{pair}{qs}{qi}"
                            )
                            for j in range(nch):
                                c0 = 512 * j
                                w = min(512, klen - c0)
                                pss = {}
                                for h in range(2):
                                    ps = mm_ps.tile(
                                        [128, 512], F32, tag="mm", name=f"sc{h}"
                                    )
                                    pb = 64 * h
                                    nc.tensor.matmul(
                                        ps[:, 0:w],
                                        QT[pair][pb:pb + 64, qstart:qstart + 128],
                                        KT[pair][pb:pb + 64, c0:c0 + w],
                                        start=True,
                                        stop=True,
                                    )
                                    pss[h] = ps
                                if general:
                                    mk = mk_pool.tile([128, 512], F32, tag="mk", name="mk")
                                    nc.sync.dma_start(
                                        out=mk[:, 0:w],
                                        in_=amask[qstart:qstart + 128, c0:c0 + w],
                                    )
                                for h in range(2):
                                    ps = pss[h]
                                    if general:
                                        nc.vector.tensor_add(ps[:, 0:w], ps[:, 0:w], mk[:, 0:w])
                                    elif j == nch - 1:
                                        nc.vector.tensor_add(
                                            ps[:, w - 128:w], ps[:, w - 128:w], causal_sb
                                        )
                                    nc.scalar.activation(
                                        A[(h, qi)][:, c0:c0 + w],
                                        ps[:, 0:w],
                                        AF.Exp,
                                        scale=0.125,
                                        accum_out=sums[:, h, j:j + 1],
                                    )
                            # normalize + write attn rows (batched stats ops)
                            rr = small.tile([128, 2], F32, tag="rr", name="rr")
                            if nch > 1:
                                s2 = small.tile([128, 2], F32, tag="s2", name="s2")
                                nc.vector.reduce_sum(
                                    s2, sums[:, :, 0:nch], axis=mybir.AxisListType.X
                                )
                                nc.vector.reciprocal(rr, s2)
                            else:
                                nc.vector.reciprocal(rr, sums[:, :, 0])
                            for h in range(2):
                                at = A[(h, qi)]
                                nc.vector.tensor_scalar_mul(
                                    at[:, 0:klen], at[:, 0:klen], rr[:, h:h + 1]
                                )
                                h_loc = 2 * pair + h
                                nc.sync.dma_start(
                                    out=attn_p[h_loc, qstart:qstart + 128, 0:klen],
                                    in_=at[:, 0:klen].bitcast(F32),
                                )

                        if ablate in ("scores",):
                            continue
                        # --- context: transpose P, col-tiled matmuls ---
                        ctx = ctx_ps_pool.tile([128, 512], F32, tag="ctx", name="ctx")
                        nkb = 16 if general else 4 * qs + 4
                        for kb in range(nkb):
                            jstart = 0 if general else max(0, kb - 4 * qs)
                            for h in range(2):
                                ptp = pt_ps_pool.tile(
                                    [128, 512], F32R, tag="pt", name="ptps"
                                )
                                for qi in range(jstart, 4):
                                    nc.tensor.transpose(
                                        ptp[:, 128 * qi:128 * qi + 128],
                                        A[(h, qi)][:, 128 * kb:128 * kb + 128],
                                        idt,
                                    )
                                pts = pt_pool.tile([128, 512], F16, tag="pts", name="pts")
                                pcopy(pts[:, 128 * jstart:512], ptp[:, 128 * jstart:512])
                                nc.tensor.matmul(
                                    ctx[64 * h:64 * h + 64, 128 * jstart:512],
                                    v_sb[:, kb, 128 * pair + 64 * h:128 * pair + 64 * h + 64],
                                    pts[:, 128 * jstart:512],
                                    start=(kb == 0),
                                    stop=(kb == nkb - 1),
                                )
                        pcopy(ctxT[pair][:, 512 * qs:512 * qs + 512], ctx)

                # ---- fc partials (row-parallel) ----
                for qt in range(0 if ablate else 16):
                    for nt in range(2):
                        ps = mm_ps.tile([128, 512], F32, tag="mm", name="fc_ps")
                        for pair in range(2):
                            nc.tensor.matmul(
                                ps,
                                ctxT[pair][:, 128 * qt:128 * qt + 128],
                                wo_sb[:, pair, 512 * nt:512 * nt + 512],
                                start=(pair == 0),
                                stop=(pair == 1),
                            )
                        fco = pt_pool.tile([128, 512], F32, tag="fco", name="fco")
                        pcopy(fco, ps)
                        nc.sync.dma_start(
                            out=rs_in[128 * qt:128 * qt + 128, 512 * nt:512 * nt + 512],
                            in_=fco,
                        )

            # ---- phase 3: reduce-scatter over the 4-core batch group ----
            if ablate:
                pass
            elif single:
                nc.sync.dma_start(out=rs_out[:], in_=rs_in[0:512, :])
            else:
                nc.gpsimd.collective_compute(
                    "ReduceScatter",
                    OP.add,
                    replica_groups=[[0, 1, 2, 3], [4, 5, 6, 7]],
                    ins=[rs_in[:]],
                    outs=[rs_out[:]],
                )

            # ---- phase 4: residual + layernorm on this core's quarter ----
            with tc.tile_pool(name="ln", bufs=2) as ln_pool:
                for qt in range(0 if ablate else 4):
                    fc_sb = ln_pool.tile([128, D], F32, tag="fc", name="fc_sb")
                    nc.sync.dma_start(
                        out=fc_sb, in_=rs_out[128 * qt:128 * qt + 128, :]
                    )
                    x_sb = ln_pool.tile([128, D], F32, tag="x", name="x_sb")
                    nc.sync.dma_start(out=x_sb, in_=x_res[128 * qt:128 * qt + 128, :])
                    t = ln_pool.tile([128, D], F32, tag="t", name="t_sb")
                    nc.vector.tensor_add(t, fc_sb, x_sb)
                    st = small.tile([128, 2, 6], F32, tag="bnst", name="bnst")
                    for sub in range(2):
                        nc.vector.bn_stats(st[:, sub, :], t[:, 512 * sub:512 * sub + 512])
                    mv = small.tile([128, 2], F32, tag="mv", name="mv")
                    nc.vector.bn_aggr(mv, st)
                    sd = small.tile([128, 1], F32, tag="sd", name="sd")
                    nc.scalar.activation(
                        sd, mv[:, 1:2], AF.Sqrt, bias=eps_sb, scale=1.0
                    )
                    rstd = small.tile([128, 1], F32, tag="rstd", name="rstd")
                    nc.vector.reciprocal(rstd, sd)
                    nc.vector.tensor_scalar(
                        t, t, mv[:, 0:1], rstd, OP.subtract, OP.mult
                    )
                    nc.vector.tensor_mul(t, t, gamma_bc)
                    nc.vector.tensor_add(t, t, beta_bc)
                    nc.sync.dma_start(
                        out=out_p[128 * qt:128 * qt + 128, :], in_=t
                    )

    nc.compile()
    return nc


def _get_nc(general: bool):
    key = ("gen" if general else "causal")
    if key not in _cache:
        _cache[key] = _build(general)
    return _cache[key]


def _make_in_maps(input_Q, input_K, input_V, W_Q, W_K, W_V, W_O,
                  ln_gamma, ln_beta, amask_f32=None):
    tri = np.triu(np.full((128, 128), NEG, dtype=np.float32), k=1)
    eye = np.eye(128, dtype=np.float32)
    in_maps = []
    for c in range(N_CORES):
        b, hg = divmod(c, 4)
        m = {
            "xt_q": np.ascontiguousarray(input_Q[b].T).astype(np.float16),
            "xt_k": np.ascontiguousarray(input_K[b].T).astype(np.float16),
            "xt_v": np.ascontiguousarray(input_V[b].T).astype(np.float16),
            "x_res": np.ascontiguousarray(input_Q[b, 512 * hg:512 * hg + 512, :]),
            "w_q": np.ascontiguousarray(W_Q[:, 256 * hg:256 * hg + 256]).astype(np.float16),
            "w_k": np.ascontiguousarray(W_K[:, 256 * hg:256 * hg + 256]).astype(np.float16),
            "w_v": np.ascontiguousarray(W_V[:, 256 * hg:256 * hg + 256]).astype(np.float16),
            "wo_loc": np.ascontiguousarray(W_O[256 * hg:256 * hg + 256, :]).astype(np.float16),
            "gamma": ln_gamma,
            "beta": ln_beta,
            "causal": tri,
            "ident": eye,
        }
        if amask_f32 is not None:
            m["amask"] = amask_f32[b]
        in_maps.append(m)
    return in_maps


def kernel(input_Q, input_K, input_V, attn_mask, W_Q, W_K, W_V, W_O, ln_gamma, ln_beta):
    from concourse.bass_utils import run_bass_kernel_spmd

    input_Q = np.asarray(input_Q, dtype=np.float32)
    input_K = np.asarray(input_K, dtype=np.float32)
    input_V = np.asarray(input_V, dtype=np.float32)
    attn_mask = np.asarray(attn_mask)
    W_Q = np.asarray(W_Q, dtype=np.float32)
    W_K = np.asarray(W_K, dtype=np.float32)
    W_V = np.asarray(W_V, dtype=np.float32)
    W_O = np.asarray(W_O, dtype=np.float32)
    ln_gamma = np.asarray(ln_gamma, dtype=np.float32)
    ln_beta = np.asarray(ln_beta, dtype=np.float32)

    causal_ref = np.triu(np.ones((S, S), dtype=bool), k=1)
    general = not all(np.array_equal(attn_mask[b], causal_ref) for b in range(B))

    nc = _get_nc(general)
    amask_f32 = None
    if general:
        amask_f32 = np.where(attn_mask, np.float32(NEG), np.float32(0.0))
    in_maps = _make_in_maps(input_Q, input_K, input_V, W_Q, W_K, W_V, W_O,
                            ln_gamma, ln_beta, amask_f32)

    res = run_bass_kernel_spmd(nc, in_maps, core_ids=list(range(N_CORES)))
    kernel.last_results = res
    kernel.last_in_maps = in_maps

    out = np.empty((B, S, D), dtype=np.float32)
    attn = np.empty((B, H, S, S), dtype=np.float32)
    for c in range(N_CORES):
        b, hg = divmod(c, 4)
        attn[b, 4 * hg:4 * hg + 4] = res.results[c]["attn_p"]
        out[b, 512 * hg:512 * hg + 512, :] = res.results[c]["out_p"]
    return out, attn


# revision 10
# speedup vs baseline: 25169.0579x; 1.0308x over previous
import sys

sys.path.insert(0, "/opt/trn_rl_repo")

import numpy as np

B, S, D, H, DK, DV = 2, 2048, 1024, 16, 64, 64
N_CORES = 8
LN_EPS = 1e-5
NEG = -1.0e9

_cache = {}


def _build(general: bool, single: bool = False, ablate: str = ""):
    import concourse.bacc as bacc
    import concourse.bass as bass
    import concourse.tile as tile
    from concourse import mybir

    F32 = mybir.dt.float32
    F32R = mybir.dt.float32r
    F16 = mybir.dt.float16
    AF = mybir.ActivationFunctionType
    OP = mybir.AluOpType

    nc = bacc.Bacc("TRN2", target_bir_lowering=False, debug=False,
                   num_devices=(1 if single else N_CORES))

    # fp16 inputs for all matmul operands (10-bit mantissa ~= tf32 precision)
    xt_q = nc.dram_tensor("xt_q", [D, S], F16, kind="ExternalInput")
    xt_k = nc.dram_tensor("xt_k", [D, S], F16, kind="ExternalInput")
    xt_v = nc.dram_tensor("xt_v", [D, S], F16, kind="ExternalInput")
    x_res = nc.dram_tensor("x_res", [512, D], F32, kind="ExternalInput")
    w_q = nc.dram_tensor("w_q", [D, 256], F16, kind="ExternalInput")
    w_k = nc.dram_tensor("w_k", [D, 256], F16, kind="ExternalInput")
    w_v = nc.dram_tensor("w_v", [D, 256], F16, kind="ExternalInput")
    wo_loc = nc.dram_tensor("wo_loc", [256, D], F16, kind="ExternalInput")
    gamma = nc.dram_tensor("gamma", [D], F32, kind="ExternalInput")
    beta = nc.dram_tensor("beta", [D], F32, kind="ExternalInput")
    causal = nc.dram_tensor("causal", [128, 128], F32, kind="ExternalInput")
    ident = nc.dram_tensor("ident", [128, 128], F32, kind="ExternalInput")
    if general:
        amask = nc.dram_tensor("amask", [S, S], F32, kind="ExternalInput")

    attn_p = nc.dram_tensor("attn_p", [4, S, S], F32, kind="ExternalOutput")
    out_p = nc.dram_tensor("out_p", [512, D], F32, kind="ExternalOutput")

    rs_in = nc.dram_tensor("rs_in", [S, D], F32)
    rs_out = nc.dram_tensor("rs_out", [512, D], F32)

    with tile.TileContext(nc) as tc:
        with tc.tile_pool(name="const", bufs=1) as const_pool, \
             tc.tile_pool(name="qtkt", bufs=1) as qt_pool, \
             tc.tile_pool(name="vp", bufs=1) as v_pool, \
             tc.tile_pool(name="small", bufs=8) as small, \
             tc.tile_pool(name="mm_ps", bufs=2, space="PSUM") as mm_ps, \
             tc.tile_pool(name="ctx_ps", bufs=2, space="PSUM") as ctx_ps_pool, \
             tc.tile_pool(name="pt_ps", bufs=2, space="PSUM") as pt_ps_pool:

            # ---- constants ----
            idt = const_pool.tile([128, 128], F32R)
            nc.sync.dma_start(out=idt, in_=ident[:].bitcast(F32R))
            causal_sb = const_pool.tile([128, 128], F32)
            nc.sync.dma_start(out=causal_sb, in_=causal[:])
            gamma_bc = const_pool.tile([128, D], F32)
            nc.gpsimd.dma_start(
                out=gamma_bc,
                in_=bass.AP(tensor=gamma[:].tensor, offset=0, ap=[[0, 128], [1, D]]),
            )
            beta_bc = const_pool.tile([128, D], F32)
            nc.gpsimd.dma_start(
                out=beta_bc,
                in_=bass.AP(tensor=beta[:].tensor, offset=0, ap=[[0, 128], [1, D]]),
            )
            eps_sb = const_pool.tile([128, 1], F32)
            nc.vector.memset(eps_sb, LN_EPS)

            # ---- phase 1: projections ----
            QT = {}
            KT = {}
            with tc.tile_pool(name="xt", bufs=10) as xt_pool, \
                 tc.tile_pool(name="wp", bufs=3) as w_pool:
                for tgt, xt_dram, w_dram in (("q", xt_q, w_q), ("k", xt_k, w_k)):
                    w_sb = w_pool.tile([128, 8, 256], F16, tag="w", name=f"w_{tgt}")
                    for ci in range(8):
                        nc.sync.dma_start(
                            out=w_sb[:, ci, :],
                            in_=w_dram[128 * ci:128 * ci + 128, :],
                        )
                    xch = []
                    for ci in range(8):
                        xc = xt_pool.tile([128, S], F16, tag="xt", name=f"x_{tgt}{ci}")
                        nc.sync.dma_start(
                            out=xc, in_=xt_dram[128 * ci:128 * ci + 128, :]
                        )
                        xch.append(xc)
                    for pair in range(2):
                        dstt = qt_pool.tile([128, S], F16, name=f"{tgt}t_{pair}")
                        for q2 in range(2):
                            ps = mm_ps.tile([128, 1024], F32, tag="mm", name="proj_ps")
                            for half in range(2):
                                c0 = 1024 * q2 + 512 * half
                                for ci in range(8):
                                    nc.tensor.matmul(
                                        ps[:, 512 * half:512 * half + 512],
                                        w_sb[:, ci, 128 * pair:128 * pair + 128],
                                        xch[ci][:, c0:c0 + 512],
                                        start=(ci == 0),
                                        stop=(ci == 7),
                                    )
                            if q2 % 2 == 0:
                                nc.vector.tensor_copy(dstt[:, 1024 * q2:1024 * q2 + 1024], ps)
                            else:
                                nc.scalar.copy(dstt[:, 1024 * q2:1024 * q2 + 1024], ps)
                        if tgt == "q":
                            QT[pair] = dstt
                        else:
                            KT[pair] = dstt

                # V in natural [s, d_v*4] layout
                wv_sb = w_pool.tile([128, 8, 256], F16, tag="w", name="w_v_sb")
                for ci in range(8):
                    nc.sync.dma_start(
                        out=wv_sb[:, ci, :],
                        in_=w_v[128 * ci:128 * ci + 128, :],
                    )
                xchv = []
                for ci in range(8):
                    xc = xt_pool.tile([128, S], F16, tag="xt", name=f"x_v{ci}")
                    nc.sync.dma_start(
                        out=xc, in_=xt_v[128 * ci:128 * ci + 128, :]
                    )
                    xchv.append(xc)
                v_sb = v_pool.tile([128, 16, 256], F16)
                for st in range(16):
                    ps = mm_ps.tile([128, 256], F32, tag="mm", name="v_ps")
                    for ci in range(8):
                        nc.tensor.matmul(
                            ps,
                            xchv[ci][:, 128 * st:128 * st + 128],
                            wv_sb[:, ci, :],
                            start=(ci == 0),
                            stop=(ci == 7),
                        )
                    if st % 2 == 0:
                        nc.vector.tensor_copy(v_sb[:, st, :], ps)
                    else:
                        nc.scalar.copy(v_sb[:, st, :], ps)

            # ---- phase 2: attention ----
            with tc.tile_pool(name="ap", bufs=12) as a_pool, \
                 tc.tile_pool(name="ptp", bufs=4) as pt_pool, \
                 tc.tile_pool(name="ctxsb", bufs=1) as ctx_sb_pool, \
                 tc.tile_pool(name="wo", bufs=1) as wo_pool, \
                 tc.tile_pool(name="mk", bufs=3) as mk_pool:

                wo_sb = wo_pool.tile([128, 2, D], F16)
                for pair in range(2):
                    nc.sync.dma_start(
                        out=wo_sb[:, pair, :],
                        in_=wo_loc[128 * pair:128 * pair + 128, :],
                    )

                ctxT = {}
                for pair in range(2):
                    ctxT[pair] = ctx_sb_pool.tile([128, S], F16, name=f"ctxT_{pair}")

                cpy = [0]

                def pcopy(dst, src):
                    # PSUM->SBUF copies: 3/5 on DVE, 2/5 on ACT (ACT is busier)
                    if cpy[0] % 5 in (0, 2, 4):
                        nc.vector.tensor_copy(dst, src)
                    else:
                        nc.scalar.copy(dst, src)
                    cpy[0] += 1

                for pair in range(0 if ablate == "proj" else 2):
                    for qs in range(4):
                        A = {}
                        # --- scores + softmax for the 4 q-subtiles ---
                        for qi in range(4):
                            qstart = 512 * qs + 128 * qi
                            klen = S if general else qstart + 128
                            nch = (klen + 511) // 512
                            A[(0, qi)] = a_pool.tile(
                                [128, 2048], F32R, tag="a", name=f"a{pair}{qs}{qi}0"
                            )
                            A[(1, qi)] = a_pool.tile(
                                [128, 2048], F32R, tag="a", name=f"a{pair}{qs}{qi}1"
                            )
                            nch = (klen + 1023) // 1024
                            sums = small.tile(
                                [128, 2, 2], F32, tag="sums", name=f"s{pair}{qs}{qi}"
                            )
                            for j in range(nch):
                                c0 = 1024 * j
                                w = min(1024, klen - c0)
                                pss = {}
                                for h in range(2):
                                    ps = mm_ps.tile(
                                        [128, 1024], F32, tag="mm", name=f"sc{h}"
                                    )
                                    pb = 64 * h
                                    for sub in range(0, w, 512):
                                        sw = min(512, w - sub)
                                        nc.tensor.matmul(
                                            ps[:, sub:sub + sw],
                                            QT[pair][pb:pb + 64, qstart:qstart + 128],
                                            KT[pair][pb:pb + 64, c0 + sub:c0 + sub + sw],
                                            start=True,
                                            stop=True,
                                        )
                                    pss[h] = ps
                                if general:
                                    mk = mk_pool.tile([128, 1024], F32, tag="mk", name="mk")
                                    nc.sync.dma_start(
                                        out=mk[:, 0:w],
                                        in_=amask[qstart:qstart + 128, c0:c0 + w],
                                    )
                                for h in range(2):
                                    ps = pss[h]
                                    if general:
                                        nc.vector.tensor_add(ps[:, 0:w], ps[:, 0:w], mk[:, 0:w])
                                    elif j == nch - 1:
                                        nc.vector.tensor_add(
                                            ps[:, w - 128:w], ps[:, w - 128:w], causal_sb
                                        )
                                    nc.scalar.activation(
                                        A[(h, qi)][:, c0:c0 + w],
                                        ps[:, 0:w],
                                        AF.Exp,
                                        scale=0.125,
                                        accum_out=sums[:, h, j:j + 1],
                                    )
                            # normalize + write attn rows (batched stats ops)
                            rr = small.tile([128, 2], F32, tag="rr", name="rr")
                            if nch > 1:
                                s2 = small.tile([128, 2], F32, tag="s2", name="s2")
                                nc.vector.reduce_sum(
                                    s2, sums[:, :, 0:nch], axis=mybir.AxisListType.X
                                )
                                nc.vector.reciprocal(rr, s2)
                            else:
                                nc.vector.reciprocal(rr, sums[:, :, 0])
                            for h in range(2):
                                at = A[(h, qi)]
                                nc.vector.tensor_scalar_mul(
                                    at[:, 0:klen], at[:, 0:klen], rr[:, h:h + 1]
                                )
                                h_loc = 2 * pair + h
                                nc.sync.dma_start(
                                    out=attn_p[h_loc, qstart:qstart + 128, 0:klen],
                                    in_=at[:, 0:klen].bitcast(F32),
                                )

                        if ablate in ("scores",):
                            continue
                        # --- context: transpose P, col-tiled matmuls ---
                        ctx = ctx_ps_pool.tile([128, 512], F32, tag="ctx", name="ctx")
                        nkb = 16 if general else 4 * qs + 4
                        for kb in range(nkb):
                            jstart = 0 if general else max(0, kb - 4 * qs)
                            for h in range(2):
                                ptp = pt_ps_pool.tile(
                                    [128, 512], F32R, tag="pt", name="ptps"
                                )
                                for qi in range(jstart, 4):
                                    nc.tensor.transpose(
                                        ptp[:, 128 * qi:128 * qi + 128],
                                        A[(h, qi)][:, 128 * kb:128 * kb + 128],
                                        idt,
                                    )
                                pts = pt_pool.tile([128, 512], F16, tag="pts", name="pts")
                                pcopy(pts[:, 128 * jstart:512], ptp[:, 128 * jstart:512])
                                nc.tensor.matmul(
                                    ctx[64 * h:64 * h + 64, 128 * jstart:512],
                                    v_sb[:, kb, 128 * pair + 64 * h:128 * pair + 64 * h + 64],
                                    pts[:, 128 * jstart:512],
                                    start=(kb == 0),
                                    stop=(kb == nkb - 1),
                                )
                        pcopy(ctxT[pair][:, 512 * qs:512 * qs + 512], ctx)

                # ---- fc partials (row-parallel) ----
                for qt in range(0 if ablate else 16):
                    ps = mm_ps.tile([128, 1024], F32, tag="mm", name="fc_ps")
                    for nt in range(2):
                        for pair in range(2):
                            nc.tensor.matmul(
                                ps[:, 512 * nt:512 * nt + 512],
                                ctxT[pair][:, 128 * qt:128 * qt + 128],
                                wo_sb[:, pair, 512 * nt:512 * nt + 512],
                                start=(pair == 0),
                                stop=(pair == 1),
                            )
                    fco = pt_pool.tile([128, 1024], F32, tag="fco", name="fco")
                    pcopy(fco, ps)
                    nc.sync.dma_start(
                        out=rs_in[128 * qt:128 * qt + 128, :],
                        in_=fco,
                    )

            # ---- phase 3: reduce-scatter over the 4-core batch group ----
            if ablate:
                pass
            elif single:
                nc.sync.dma_start(out=rs_out[:], in_=rs_in[0:512, :])
            else:
                nc.gpsimd.collective_compute(
                    "ReduceScatter",
                    OP.add,
                    replica_groups=[[0, 1, 2, 3], [4, 5, 6, 7]],
                    ins=[rs_in[:]],
                    outs=[rs_out[:]],
                )

            # ---- phase 4: residual + layernorm on this core's quarter ----
            with tc.tile_pool(name="ln", bufs=2) as ln_pool:
                for qt in range(0 if ablate else 4):
                    fc_sb = ln_pool.tile([128, D], F32, tag="fc", name="fc_sb")
                    nc.sync.dma_start(
                        out=fc_sb, in_=rs_out[128 * qt:128 * qt + 128, :]
                    )
                    x_sb = ln_pool.tile([128, D], F32, tag="x", name="x_sb")
                    nc.sync.dma_start(out=x_sb, in_=x_res[128 * qt:128 * qt + 128, :])
                    t = ln_pool.tile([128, D], F32, tag="t", name="t_sb")
                    nc.vector.tensor_add(t, fc_sb, x_sb)
                    st = small.tile([128, 2, 6], F32, tag="bnst", name="bnst")
                    for sub in range(2):
                        nc.vector.bn_stats(st[:, sub, :], t[:, 512 * sub:512 * sub + 512])
                    mv = small.tile([128, 2], F32, tag="mv", name="mv")
                    nc.vector.bn_aggr(mv, st)
                    sd = small.tile([128, 1], F32, tag="sd", name="sd")
                    nc.scalar.activation(
                        sd, mv[:, 1:2], AF.Sqrt, bias=eps_sb, scale=1.0
                    )
                    rstd = small.tile([128, 1], F32, tag="rstd", name="rstd")
                    nc.vector.reciprocal(rstd, sd)
                    nc.vector.tensor_scalar(
                        t, t, mv[:, 0:1], rstd, OP.subtract, OP.mult
                    )
                    nc.vector.tensor_mul(t, t, gamma_bc)
                    nc.vector.tensor_add(t, t, beta_bc)
                    nc.sync.dma_start(
                        out=out_p[128 * qt:128 * qt + 128, :], in_=t
                    )

    nc.compile()
    return nc


def _get_nc(general: bool):
    key = ("gen" if general else "causal")
    if key not in _cache:
        _cache[key] = _build(general)
    return _cache[key]


def _make_in_maps(input_Q, input_K, input_V, W_Q, W_K, W_V, W_O,
                  ln_gamma, ln_beta, amask_f32=None):
    tri = np.triu(np.full((128, 128), NEG, dtype=np.float32), k=1)
    eye = np.eye(128, dtype=np.float32)
    in_maps = []
    for c in range(N_CORES):
        b, hg = divmod(c, 4)
        m = {
            "xt_q": np.ascontiguousarray(input_Q[b].T).astype(np.float16),
            "xt_k": np.ascontiguousarray(input_K[b].T).astype(np.float16),
            "xt_v": np.ascontiguousarray(input_V[b].T).astype(np.float16),
            "x_res": np.ascontiguousarray(input_Q[b, 512 * hg:512 * hg + 512, :]),
            "w_q": np.ascontiguousarray(W_Q[:, 256 * hg:256 * hg + 256]).astype(np.float16),
            "w_k": np.ascontiguousarray(W_K[:, 256 * hg:256 * hg + 256]).astype(np.float16),
            "w_v": np.ascontiguousarray(W_V[:, 256 * hg:256 * hg + 256]).astype(np.float16),
            "wo_loc": np.ascontiguousarray(W_O[256 * hg:256 * hg + 256, :]).astype(np.float16),
            "gamma": ln_gamma,
            "beta": ln_beta,
            "causal": tri,
            "ident": eye,
        }
        if amask_f32 is not None:
            m["amask"] = amask_f32[b]
        in_maps.append(m)
    return in_maps


def kernel(input_Q, input_K, input_V, attn_mask, W_Q, W_K, W_V, W_O, ln_gamma, ln_beta):
    from concourse.bass_utils import run_bass_kernel_spmd

    input_Q = np.asarray(input_Q, dtype=np.float32)
    input_K = np.asarray(input_K, dtype=np.float32)
    input_V = np.asarray(input_V, dtype=np.float32)
    attn_mask = np.asarray(attn_mask)
    W_Q = np.asarray(W_Q, dtype=np.float32)
    W_K = np.asarray(W_K, dtype=np.float32)
    W_V = np.asarray(W_V, dtype=np.float32)
    W_O = np.asarray(W_O, dtype=np.float32)
    ln_gamma = np.asarray(ln_gamma, dtype=np.float32)
    ln_beta = np.asarray(ln_beta, dtype=np.float32)

    causal_ref = np.triu(np.ones((S, S), dtype=bool), k=1)
    general = not all(np.array_equal(attn_mask[b], causal_ref) for b in range(B))

    nc = _get_nc(general)
    amask_f32 = None
    if general:
        amask_f32 = np.where(attn_mask, np.float32(NEG), np.float32(0.0))
    in_maps = _make_in_maps(input_Q, input_K, input_V, W_Q, W_K, W_V, W_O,
                            ln_gamma, ln_beta, amask_f32)

    res = run_bass_kernel_spmd(nc, in_maps, core_ids=list(range(N_CORES)))
    kernel.last_results = res
    kernel.last_in_maps = in_maps

    out = np.empty((B, S, D), dtype=np.float32)
    attn = np.empty((B, H, S, S), dtype=np.float32)
    for c in range(N_CORES):
        b, hg = divmod(c, 4)
        attn[b, 4 * hg:4 * hg + 4] = res.results[c]["attn_p"]
        out[b, 512 * hg:512 * hg + 512, :] = res.results[c]["out_p"]
    return out, attn
